# revision 1
# baseline (speedup 1.0000x reference)
"""Trainium2 Bass kernel for nn_ConvolutionLayer (FFT conv collapse).

Math: reference computes
    u_fft = rfft(u); ev_fft = rfft(ev)
    p_fft = einsum('bi,kj->bkj', u_fft, ev_fft)      # sums u_fft over i!
    conv  = irfft(p_fft); result = einsum('bkl,k->bl', conv, lam)

The einsum has no shared index, so p_fft[b,k,j] = s_b * ev_fft[k,j] with
s_b = sum_i u_fft[b,i] = u[b,:] @ g   (g = fft(indicator of first L/2+1)).
irfft is R-linear, so with s_b = a_b + i*c_b:
    result[b,:] = a_b * w0 + c_b * w1
    w0 = lam @ ev                       (since irfft(rfft(e)) = e)
    w1 = irfft(i * rfft(w0))            (by linearity over k)
w1 is computed on-device via a 4-step Cooley-Tukey matmul-FFT (64x128),
with the Hermitian symbol (+i / -i / 0) applied in the middle.

Sharding: batch (64) across 8 cores, 8 rows each; the w0/w1 pipeline is
tiny and computed redundantly on every core (no collectives).

Device layouts (per core):
  U    (128p x 512f)   u shard, p = 16*b_loc + t, l = 512*t + f
  EVL  (128p x 2236f)  [EVr | LAMB2]:
        EVr[32s+k, 128t+b]  = ev[k, 128(4t+s)+b]
        LAMB2[32s+k, 60+s]  = LAMB2[32s+k, 124+s] = lam[k]
  xps  (128p x 128f)   [x; x] where x[a,b] = w0[128a+b], built by 16
        accumulating matmuls (lhsT = sliding LAMB2 window)
  FFT: x ->(F64)-> YT ->(*WT)-> ZT ->(F128)-> XT ->(i*sgn)-> X'T
        ->(I128)-> P ->(*Wi)-> Q ->(I64/L, doubled)-> Y2 = [w1; w1]
  final: res_j = a_b * X2 + c_b * Y2 for batch pair b = (2j, 2j+1)
        stacked on the 128 partitions.
"""

import numpy as np

_B, _K, _L = 64, 32, 8192
_NC = 8
_BS = _B // _NC  # 8 batch rows per core
_N1, _N2 = 64, 128  # l = 128*a + b

# ---------------------------------------------------------------- constants


def _build_constants():
    L, N1, N2 = _L, _N1, _N2
    ind = np.zeros(L)
    ind[: L // 2 + 1] = 1.0
    g = np.fft.fft(ind)  # g[n] = sum_{i=0}^{L/2} e^{-2pi i n i/L}

    gU_re = np.tile(g.real.astype(np.float32).reshape(16, 512), (8, 1))
    gU_im = np.tile(g.imag.astype(np.float32).reshape(16, 512), (8, 1))

    # MASK8 (128 x 8): col 2j+i live for partition groups {2j, 2j+1}
    MASK8 = np.zeros((128, 8), np.float32)
    for p in range(128):
        j = (p // 16) // 2
        MASK8[p, 2 * j : 2 * j + 2] = 1.0
    # STK (128 x 128): STK[p, m] = ((p//16) % 2 == m//64)
    STK = np.zeros((128, 128), np.float32)
    for p in range(128):
        STK[p, 64 * ((p // 16) % 2) : 64 * ((p // 16) % 2) + 64] = 1.0

    a_i = np.arange(N1)
    b_i = np.arange(N2)
    F64 = np.exp(-2j * np.pi * np.outer(a_i, a_i) / N1)
    WT = np.exp(-2j * np.pi * np.outer(b_i, a_i) / L)
    F128 = np.exp(-2j * np.pi * np.outer(b_i, b_i) / N2)
    k = a_i[None, :] + N1 * b_i[:, None]  # (128d, 64c)
    sgnT = np.where(
        (k >= 1) & (k <= L // 2 - 1), 1.0, np.where(k > L // 2, -1.0, 0.0)
    )
    I128 = np.exp(+2j * np.pi * np.outer(b_i, b_i) / N2)
    Wi = np.exp(+2j * np.pi * np.outer(a_i, b_i) / L)
    I64s = np.exp(+2j * np.pi * np.outer(a_i, a_i) / N1) / L
    I64re2 = np.hstack([I64s.real, I64s.real])  # (64 x 128) doubled
    I64imN2 = np.hstack([-I64s.imag, -I64s.imag])

    f32 = lambda x: np.ascontiguousarray(np.asarray(x, np.float32))

    p128 = {
        "gU_re": f32(gU_re),
        "gU_im": f32(gU_im),
        "MASK8": f32(MASK8),
        "STK": f32(STK),
        "WT_re": f32(WT.real),
        "WT_im": f32(WT.imag),
        "F128_re": f32(F128.real),
        "F128_im": f32(F128.imag),
        "F128_imN": f32(-F128.imag),
        "sgnT": f32(sgnT),
        "sgnTN": f32(-sgnT),
        "I128_re": f32(I128.real),
        "I128_im": f32(I128.imag),
        "I128_imN": f32(-I128.imag),
    }
    p64 = {
        "F64_re": f32(F64.real),
        "F64_im": f32(F64.imag),
        "Wi_re": f32(Wi.real),
        "Wi_im": f32(Wi.imag),
        "I64_re2": f32(I64re2),
        "I64_imN2": f32(I64imN2),
    }

    def pack(parts):
        off, offs = 0, {}
        for name, arr in parts.items():
            offs[name] = (off, arr.shape[1])
            off += arr.shape[1]
        return np.concatenate(list(parts.values()), axis=1), offs

    CP, cp_off = pack(p128)
    CQ, cq_off = pack(p64)
    return CP, cp_off, CQ, cq_off


_CP, _CP_OFF, _CQ, _CQ_OFF = _build_constants()
_EVL_W = 2048 + 188

# ---------------------------------------------------------------- bass build

_COMPILED = None


def _build_nc():
    import concourse.mybir as mybir
    import concourse.tile as tile
    from concourse import bacc

    f32 = mybir.dt.float32
    Alu = mybir.AluOpType

    nc = bacc.Bacc(None)

    u_d = nc.declare_dram_parameter("u", [128, 512], f32, isOutput=False)
    evl_d = nc.declare_dram_parameter("evl", [128, _EVL_W], f32, isOutput=False)
    cp_d = nc.declare_dram_parameter("cp", list(_CP.shape), f32, isOutput=False)
    cq_d = nc.declare_dram_parameter("cq", list(_CQ.shape), f32, isOutput=False)
    out_d = nc.declare_dram_parameter("out", [_BS, 64, 128], f32, isOutput=True)

    def cp(t, name):
        off, w = _CP_OFF[name]
        return t[:, off : off + w]

    def cq(t, name):
        off, w = _CQ_OFF[name]
        return t[:, off : off + w]

    with tile.TileContext(nc) as tc:
        with (
            tc.tile_pool(name="const", bufs=1) as constp,
            tc.tile_pool(name="sb", bufs=1) as sb,
            tc.tile_pool(name="work", bufs=2) as work,
            tc.tile_pool(name="res", bufs=3) as resp,
            tc.tile_pool(name="psx", bufs=1, space="PSUM") as psx,
            tc.tile_pool(name="psacr", bufs=1, space="PSUM") as psacr,
            tc.tile_pool(name="psfft", bufs=2, space="PSUM") as psfft,
            tc.tile_pool(name="psy", bufs=1, space="PSUM") as psy,
        ):
            EVL = constp.tile([128, _EVL_W], f32)
            U = constp.tile([128, 512], f32)
            CP = constp.tile([128, _CP.shape[1]], f32)
            CQ = constp.tile([64, _CQ.shape[1]], f32)
            nc.sync.dma_start(EVL[:], evl_d[:])
            nc.sync.dma_start(U[:], u_d[:])
            nc.sync.dma_start(CP[:], cp_d[:])
            nc.sync.dma_start(CQ[:], cq_d[:])

            # ---- PE first: xps = [x; x], 16 accumulating matmuls --------
            xps = psx.tile([128, 128], f32)
            for t in range(16):
                nc.tensor.matmul(
                    xps[:],
                    EVL[:, 2048 + 60 - 4 * t : 2048 + 188 - 4 * t],
                    EVL[:, 128 * t : 128 * t + 128],
                    start=(t == 0),
                    stop=(t == 15),
                )

            # ---- DVE: fused per-partition dots  U.g_re, U.g_im ----------
            scratch = sb.tile([128, 512], f32)
            R = sb.tile([128, 2], f32)
            nc.vector.scalar_tensor_tensor(
                scratch[:], U[:], 1.0, cp(CP, "gU_re"),
                op0=Alu.mult, op1=Alu.mult, accum_out=R[:, 0:1],
            )
            nc.vector.scalar_tensor_tensor(
                scratch[:], U[:], 1.0, cp(CP, "gU_im"),
                op0=Alu.mult, op1=Alu.mult, accum_out=R[:, 1:2],
            )
            # R2p (128 x (4j,2i)) = MASK8 * broadcast(R)
            R2p = sb.tile([128, 8], f32)
            nc.vector.tensor_tensor(
                R2p[:].rearrange("p (j i) -> p j i", i=2),
                cp(CP, "MASK8").rearrange("p (j i) -> p j i", i=2),
                R[:].unsqueeze(1).broadcast_to((128, 4, 2)),
                Alu.mult,
            )
            # acrP[m, 2j+i] = ac[2j + m//64, i]  (paired-batch scalars)
            acrP = psacr.tile([128, 8], f32)
            nc.tensor.matmul(acrP[:], cp(CP, "STK"), R2p[:], start=True, stop=True)

            # X2 = [x; x] in SBUF (PE lhsT source + final-stage operand)
            X2 = sb.tile([128, 128], f32)
            nc.vector.tensor_copy(X2[:], xps[:])

            # ---- FFT stage 1: YT[b,c] = sum_a x[a,b] F64[a,c] -----------
            YTre = psfft.tile([128, 64], f32, tag="fftA")
            YTim = psfft.tile([128, 64], f32, tag="fftB")
            nc.tensor.matmul(YTre[:], X2[0:64, :], cq(CQ, "F64_re"), start=True, stop=True)
            nc.tensor.matmul(YTim[:], X2[0:64, :], cq(CQ, "F64_im"), start=True, stop=True)

            # ---- twiddle: ZT = YT * WT (complex) ------------------------
            ZTre = work.tile([128, 64], f32, tag="zt")
            ZTim = work.tile([128, 64], f32, tag="zt2")
            t1 = work.tile([128, 64], f32, tag="t1")
            t2 = work.tile([128, 64], f32, tag="t2")
            nc.vector.tensor_tensor(ZTre[:], YTre[:], cp(CP, "WT_re"), Alu.mult)
            nc.vector.tensor_tensor(t1[:], YTim[:], cp(CP, "WT_im"), Alu.mult)
            nc.vector.tensor_tensor(ZTre[:], ZTre[:], t1[:], Alu.subtract)
            nc.vector.tensor_tensor(ZTim[:], YTre[:], cp(CP, "WT_im"), Alu.mult)
            nc.vector.tensor_tensor(t2[:], YTim[:], cp(CP, "WT_re"), Alu.mult)
            nc.vector.tensor_tensor(ZTim[:], ZTim[:], t2[:], Alu.add)

            # ---- stage 2: XT[d,c] = sum_b F128[b,d] ZT[b,c] -------------
            XTre = psfft.tile([128, 64], f32, tag="fftA")
            XTim = psfft.tile([128, 64], f32, tag="fftB")
            nc.tensor.matmul(XTre[:], cp(CP, "F128_re"), ZTre[:], start=True, stop=False)
            nc.tensor.matmul(XTre[:], cp(CP, "F128_imN"), ZTim[:], start=False, stop=True)
            nc.tensor.matmul(XTim[:], cp(CP, "F128_im"), ZTre[:], start=True, stop=False)
            nc.tensor.matmul(XTim[:], cp(CP, "F128_re"), ZTim[:], start=False, stop=True)

            # ---- symbol: X' = i * sgn * X -------------------------------
            XpTre = work.tile([128, 64], f32, tag="xp")
            XpTim = work.tile([128, 64], f32, tag="xp2")
            nc.vector.tensor_tensor(XpTre[:], XTim[:], cp(CP, "sgnTN"), Alu.mult)
            nc.vector.tensor_tensor(XpTim[:], XTre[:], cp(CP, "sgnT"), Alu.mult)

            # ---- stage 3: P[c,b] = sum_d X'T[d,c] I128[d,b] -------------
            Pre = psfft.tile([64, 128], f32, tag="fftA")
            Pim = psfft.tile([64, 128], f32, tag="fftB")
            nc.tensor.matmul(Pre[:], XpTre[:], cp(CP, "I128_re"), start=True, stop=False)
            nc.tensor.matmul(Pre[:], XpTim[:], cp(CP, "I128_imN"), start=False, stop=True)
            nc.tensor.matmul(Pim[:], XpTre[:], cp(CP, "I128_im"), start=True, stop=False)
            nc.tensor.matmul(Pim[:], XpTim[:], cp(CP, "I128_re"), start=False, stop=True)

            # ---- inverse twiddle: Q = P * Wi (complex) ------------------
            Qre = work.tile([64, 128], f32, tag="q")
            Qim = work.tile([64, 128], f32, tag="q2")
            t3 = work.tile([64, 128], f32, tag="t3")
            t4 = work.tile([64, 128], f32, tag="t4")
            nc.vector.tensor_tensor(Qre[:], Pre[:], cq(CQ, "Wi_re"), Alu.mult)
            nc.vector.tensor_tensor(t3[:], Pim[:], cq(CQ, "Wi_im"), Alu.mult)
            nc.vector.tensor_tensor(Qre[:], Qre[:], t3[:], Alu.subtract)
            nc.vector.tensor_tensor(Qim[:], Pre[:], cq(CQ, "Wi_im"), Alu.mult)
            nc.vector.tensor_tensor(t4[:], Pim[:], cq(CQ, "Wi_re"), Alu.mult)
            nc.vector.tensor_tensor(Qim[:], Qim[:], t4[:], Alu.add)

            # ---- stage 4 (doubled): Y2 = [w1grid; w1grid] ---------------
            Y2 = psy.tile([128, 128], f32)
            nc.tensor.matmul(Y2[:], cq(CQ, "I64_re2"), Qre[:], start=True, stop=False)
            nc.tensor.matmul(Y2[:], cq(CQ, "I64_imN2"), Qim[:], start=False, stop=True)

            # ---- final: paired batches, out DMAs on two queues ----------
            for j in range(_BS // 2):
                tmp = resp.tile([128, 128], f32, tag="tmp")
                nc.vector.tensor_scalar_mul(
                    tmp[:], Y2[:], acrP[:, 2 * j + 1 : 2 * j + 2]
                )
                res = resp.tile([128, 128], f32, tag="res")
                nc.vector.scalar_tensor_tensor(
                    res[:], X2[:], acrP[:, 2 * j : 2 * j + 1], tmp[:],
                    op0=Alu.mult, op1=Alu.add,
                )
                nc.sync.dma_start(out_d[2 * j], res[0:64, :])
                nc.gpsimd.dma_start(out_d[2 * j + 1], res[64:128, :])

    nc.compile()
    return nc


def _get_compiled():
    global _COMPILED
    if _COMPILED is None:
        _COMPILED = _build_nc()
    return _COMPILED


# ---------------------------------------------------------------- entry


def _make_in_maps(u, eigenvectors, eigenvalues):
    u = np.ascontiguousarray(u, np.float32)
    # pure relayout (zero flops): EVr[32s+k, 128t+b] = ev[k, 128(4t+s)+b]
    evr = (
        np.asarray(eigenvectors, np.float32)
        .reshape(_K, 16, 4, 128)
        .transpose(2, 0, 1, 3)
        .reshape(128, 2048)
    )
    lamv = np.asarray(eigenvalues, np.float32)
    lamb2 = np.zeros((128, 188), np.float32)
    for s in range(4):
        lamb2[32 * s : 32 * s + 32, 60 + s] = lamv
        lamb2[32 * s : 32 * s + 32, 124 + s] = lamv
    evl = np.ascontiguousarray(np.hstack([evr, lamb2]))

    in_maps = []
    for c in range(_NC):
        in_maps.append(
            {
                "u": u[c * _BS : (c + 1) * _BS].reshape(128, 512),
                "evl": evl,
                "cp": _CP,
                "cq": _CQ,
            }
        )
    return in_maps, None


def _gather(results):
    outs = [results[c]["out"].reshape(_BS, _L) for c in range(_NC)]
    return np.concatenate(outs, axis=0)


def kernel(u, eigenvectors, eigenvalues):
    from concourse.bass_utils import run_bass_kernel_spmd

    nc = _get_compiled()
    in_maps, _ = _make_in_maps(u, eigenvectors, eigenvalues)
    res = run_bass_kernel_spmd(nc, in_maps, core_ids=list(range(_NC)))
    return _gather(res.results)



# revision 7
# speedup vs baseline: 1.1711x; 1.1711x over previous
"""Trainium2 Bass kernel for nn_ConvolutionLayer (FFT conv collapse), v2.

Math: reference computes
    u_fft = rfft(u); ev_fft = rfft(ev)
    p_fft = einsum('bi,kj->bkj', u_fft, ev_fft)      # sums u_fft over i!
    conv  = irfft(p_fft); result = einsum('bkl,k->bl', conv, lam)

The einsum has no shared index, so p_fft[b,k,j] = s_b * ev_fft[k,j] with
s_b = sum_i u_fft[b,i] = u[b,:] @ g   (g = fft(indicator of first L/2+1)).
irfft is R-linear, so with s_b = a_b + i*c_b:
    result[b,:] = a_b * w0 + c_b * w1
    w0 = lam @ ev                       (since irfft(rfft(e)) = e)
    w1 = irfft(i * rfft(w0))            (by linearity over k)
w1 is computed on-device via a 4-step Cooley-Tukey matmul-FFT (64x128).

v2 changes vs v1 (36.4us -> target <20us):
  * all PE matmuls in bf16 (fp32 runs at 4 cyc/col in LOW_HIGH mode,
    bf16 at 1); EV is shipped bf16, halving the critical input DMA
  * the Hermitian symbol i*sgn(k) is folded into the I128 inverse-DFT
    constants (sgn(k) = sigma(d) for k = c + 64d except bins k=0,4096,
    both in column c=0 where Im(I64[:,0]) = 0 kills the leak exactly)
  * g-dot tables (512KB) replaced by: even-stride reduce + cot table
    (Re g = 1 on evens + 4096 at 0; Im g = -cot(pi n/L) on odds)
  * inputs packed into 4 DMAs on one queue (LAMB2+EV split in two so
    the w0 matmuls start at half-EV); output gathered in SBUF and
    written with 2 DMAs
  * twiddle/final vector work split across DVE / Act / Pool engines

Device layouts (per core):
  U    (128p x 512f)   u shard, p = 16*b_loc + t, l = 512*t + f
  EVr  bf16: EVr[32s+k, 128t+b] = ev[k, 128(4t+s)+b]
  LAMB2 bf16 (128 x 188): LAMB2[32s+k, 60+s] = LAMB2[.., 124+s] = lam[k]
  xps  (128p x 128f) PSUM [x; x], x[a,b] = w0[128a+b], 16 acc matmuls
  FFT: x ->(F64)-> YT ->(*WT)-> ZT ->(F128)-> XT ->(I128'')-> P
       ->(*Wi)-> Q ->(I64/L, doubled)-> Y2 = [w1; w1]
  final: res_all[:, 128j+f] = a_b*X2 + c_b*Y2 for batch pair b=(2j,2j+1)
       one [128, 512] gather, 2 output DMAs
"""

import numpy as np
import ml_dtypes

_BF16 = ml_dtypes.bfloat16
_B, _K, _L = 64, 32, 8192
_NC = 8
_BS = _B // _NC  # 8 batch rows per core
_N1, _N2 = 64, 128  # l = 128*a + b

# ---------------------------------------------------------------- constants


def _pack(parts):
    off, offs = 0, {}
    for name, arr in parts.items():
        offs[name] = (off, arr.shape[1])
        off += arr.shape[1]
    return np.concatenate(list(parts.values()), axis=1), offs


def _build_constants():
    L, N1, N2 = _L, _N1, _N2
    a_i = np.arange(N1)
    b_i = np.arange(N2)
    F64 = np.exp(-2j * np.pi * np.outer(a_i, a_i) / N1)  # (64a, 64c)
    WT = np.exp(-2j * np.pi * np.outer(b_i, a_i) / L)  # (128b, 64c)
    F128 = np.exp(-2j * np.pi * np.outer(b_i, b_i) / N2)  # (128b, 128d)
    I128 = np.exp(+2j * np.pi * np.outer(b_i, b_i) / N2)  # (128d, 128b)
    Wi = np.exp(+2j * np.pi * np.outer(a_i, b_i) / L)  # (64c, 128b)
    I64s = np.exp(+2j * np.pi * np.outer(a_i, a_i) / N1) / L  # (64c, 64a)

    # fold i*sigma(d) into I128 (Hermitian symbol; bins 0/4096 self-cancel)
    sig = np.where(b_i < 64, 1.0, -1.0)[:, None]
    I128f = 1j * sig * I128
    # stage-4 lhsT, doubled free dim -> out partitions 128 = [w1; w1]
    I64re2 = np.hstack([I64s.real, I64s.real])  # (64c, 128)
    I64imN2 = np.hstack([-I64s.imag, -I64s.imag])

    # pad 64-row consts to 128 rows (partitions 64-127 unused)
    def p64(x):
        return np.vstack([x, np.zeros_like(x)])

    bf = lambda x: np.ascontiguousarray(np.asarray(x, np.float32).astype(_BF16))
    f32 = lambda x: np.ascontiguousarray(np.asarray(x, np.float32))

    # ---- bf16 const blob (matmul operands) ----
    bparts = {
        "F64_re": bf(p64(F64.real)),
        "F64_im": bf(p64(F64.imag)),
        "F128_re": bf(F128.real),
        "F128_im": bf(F128.imag),
        "F128_imN": bf(-F128.imag),
        "I128f_re": bf(I128f.real),
        "I128f_im": bf(I128f.imag),
        "I128f_imN": bf(-I128f.imag),
        "I64_re2": bf(p64(I64re2)),
        "I64_imN2": bf(p64(I64imN2)),
    }
    CB, cb_off = _pack(bparts)

    # ---- fp32 blob (DVE/Act operands + U appended at runtime) ----
    # cot table over odd l: per partition p = 16b+t the weights depend on
    # (t, f): l = 512t + 2fo + 1 -> tile the (16, 256) table 8x
    lidx = (512 * np.arange(16)[:, None] + 2 * np.arange(256)[None, :] + 1)
    cot = -1.0 / np.tan(np.pi * lidx / L)  # (16, 256)
    COT = np.tile(cot, (8, 1))  # (128, 256)
    E0V = np.zeros((128, 1), np.float32)
    E0V[::16, 0] = 4096.0
    MASK8 = np.zeros((128, 8), np.float32)
    for p in range(128):
        j = (p // 16) // 2
        MASK8[p, 2 * j : 2 * j + 2] = 1.0
    STK = np.zeros((128, 128), np.float32)
    for p in range(128):
        STK[p, 64 * ((p // 16) % 2) : 64 * ((p // 16) % 2) + 64] = 1.0

    fparts = {
        "COT": f32(COT),
        "E0V": f32(E0V),
        "MASK8": f32(MASK8),
        "STK": f32(STK),
        "WT_re": f32(WT.real),
        "WT_im": f32(WT.imag),
        "Wi_re": f32(p64(Wi.real)),
        "Wi_im": f32(p64(Wi.imag)),
    }
    CF, cf_off = _pack(fparts)
    return CB, cb_off, CF, cf_off


_CB, _CB_OFF, _CF, _CF_OFF = _build_constants()
_A1_W = 188 + 1024  # LAMB2 + first half of EVr
_A2_W = 1024  # second half of EVr
_CF_W = _CF.shape[1]
_FP_W = 512 + _CF_W  # U + fp32 consts

# ---------------------------------------------------------------- bass build

_COMPILED = None


def _build_nc():
    import concourse.mybir as mybir
    import concourse.tile as tile
    from concourse import bacc

    f32 = mybir.dt.float32
    bf16 = mybir.dt.bfloat16
    Alu = mybir.AluOpType
    Act = mybir.ActivationFunctionType

    nc = bacc.Bacc(None)

    a1_d = nc.declare_dram_parameter("a1", [128, _A1_W], bf16, isOutput=False)
    a2_d = nc.declare_dram_parameter("a2", [128, _A2_W], bf16, isOutput=False)
    cb_d = nc.declare_dram_parameter("cb", list(_CB.shape), bf16, isOutput=False)
    fp_d = nc.declare_dram_parameter("fp", [128, _FP_W], f32, isOutput=False)
    out_d = nc.declare_dram_parameter("out", [_BS, 64, 128], f32, isOutput=True)

    def cb(t, name):
        off, w = _CB_OFF[name]
        return t[:, off : off + w]

    def cf(t, name):
        off, w = _CF_OFF[name]
        return t[:, 512 + off : 512 + off + w]

    with tile.TileContext(nc) as tc:
        with (
            tc.tile_pool(name="const", bufs=1) as constp,
            tc.tile_pool(name="work", bufs=2) as work,
            tc.tile_pool(name="psx", bufs=1, space="PSUM") as psx,
            tc.tile_pool(name="psacr", bufs=1, space="PSUM") as psacr,
            tc.tile_pool(name="psfft", bufs=2, space="PSUM") as psfft,
            tc.tile_pool(name="psy", bufs=1, space="PSUM") as psy,
        ):
            A1 = constp.tile([128, _A1_W], bf16)
            A2 = constp.tile([128, _A2_W], bf16)
            CB = constp.tile([128, _CB.shape[1]], bf16)
            FP = constp.tile([128, _FP_W], f32)
            nc.sync.dma_start(A1[:], a1_d[:])
            nc.sync.dma_start(A2[:], a2_d[:])
            nc.sync.dma_start(CB[:], cb_d[:])
            nc.sync.dma_start(FP[:], fp_d[:])
            U = FP[:, 0:512]

            # ---- PE: xps = [x; x], 16 accumulating bf16 matmuls ---------
            # lhsT = sliding LAMB2 window (A1 cols 0..187), rhs = EVr chunk
            xps = psx.tile([128, 128], f32)
            for t in range(16):
                rhs = (
                    A1[:, 188 + 128 * t : 188 + 128 * t + 128]
                    if t < 8
                    else A2[:, 128 * (t - 8) : 128 * (t - 8) + 128]
                )
                nc.tensor.matmul(
                    xps[:],
                    A1[:, 60 - 4 * t : 188 - 4 * t],
                    rhs,
                    start=(t == 0),
                    stop=(t == 15),
                )

            # ---- dots: a = even-sum + 4096*u0, c = sum_odd u*(-cot) -----
            U3 = U.rearrange("p (f e) -> p e f", e=2)  # [128, 2, 256]
            R = work.tile([128, 2], f32, tag="R")
            Rraw = work.tile([128, 2], f32, tag="Rraw")
            scratch = work.tile([128, 256], f32, tag="scr")
            nc.vector.tensor_reduce(
                Rraw[:, 0:1], U3[:, 0:1, :], axis=mybir.AxisListType.X, op=Alu.add
            )
            nc.vector.scalar_tensor_tensor(
                scratch[:],
                U3[:, 1:2, :].rearrange("p e f -> p (e f)"),
                1.0,
                cf(FP, "COT"),
                op0=Alu.mult,
                op1=Alu.mult,
                accum_out=R[:, 1:2],
            )
            # a += 4096*u[b,0]  (E0V = 4096 at p%16==0)
            nc.vector.tensor_tensor(Rraw[:, 1:2], U[:, 0:1], cf(FP, "E0V"), Alu.mult)
            nc.vector.tensor_tensor(R[:, 0:1], Rraw[:, 0:1], Rraw[:, 1:2], Alu.add)
            # R2p (128 x (4j,2i)) = MASK8 * broadcast(R)
            R2p = work.tile([128, 8], f32, tag="R2p")
            nc.vector.tensor_tensor(
                R2p[:].rearrange("p (j i) -> p j i", i=2),
                cf(FP, "MASK8").rearrange("p (j i) -> p j i", i=2),
                R[:].unsqueeze(1).broadcast_to((128, 4, 2)),
                Alu.mult,
            )
            # acrP[m, 2j+i] = ac[2j + m//64, i] (paired-batch scalars), fp32
            acrP = psacr.tile([128, 8], f32)
            nc.tensor.matmul(acrP[:], cf(FP, "STK"), R2p[:], start=True, stop=True)
            acrS = work.tile([128, 8], f32, tag="acrS")
            nc.scalar.copy(acrS[:], acrP[:])

            # ---- X2 copies: fp32 (final stage) + bf16 (FFT lhsT) --------
            X2f = work.tile([128, 128], f32, tag="X2f")
            X2b = work.tile([128, 128], bf16, tag="X2b")
            nc.scalar.copy(X2f[:], xps[:])
            nc.vector.tensor_copy(X2b[:], xps[:])

            # ---- a-halves: tmpA[:, 128j:] = a_{pair j} * X2  (Act eng) --
            tmpA = work.tile([128, 512], f32, tag="tmpA")
            for j in range(4):
                nc.scalar.activation(
                    tmpA[:, 128 * j : 128 * j + 128],
                    X2f[:],
                    Act.Copy,
                    scale=acrS[:, 2 * j : 2 * j + 1],
                )

            # ---- FFT stage 1: YT[b,c] = sum_a x[a,b] F64[a,c] -----------
            YTre = psfft.tile([128, 64], f32, tag="fftA")
            YTim = psfft.tile([128, 64], f32, tag="fftB")
            nc.tensor.matmul(YTre[:], X2b[0:64, :], cb(CB, "F64_re")[0:64, :], start=True, stop=True)
            nc.tensor.matmul(YTim[:], X2b[0:64, :], cb(CB, "F64_im")[0:64, :], start=True, stop=True)

            # ---- twiddle: ZT = YT * WT (complex) ------------------------
            # GpSimd cannot read PSUM: DVE does the re-path from PSUM,
            # Act stages YT into SBUF, GpSimd does the im-path from SBUF.
            ZTre = work.tile([128, 64], bf16, tag="zt")
            ZTim = work.tile([128, 64], bf16, tag="zt2")
            t1 = work.tile([128, 64], f32, tag="t1")
            t2 = work.tile([128, 64], f32, tag="t2")
            t3 = work.tile([128, 64], f32, tag="t3")
            t4 = work.tile([128, 64], f32, tag="t4")
            YreS = work.tile([128, 64], f32, tag="yres")
            YimS = work.tile([128, 64], f32, tag="yims")
            nc.scalar.copy(YreS[:], YTre[:])
            nc.scalar.copy(YimS[:], YTim[:])
            nc.vector.tensor_tensor(t1[:], YTre[:], cf(FP, "WT_re"), Alu.mult)
            nc.vector.tensor_tensor(t2[:], YTim[:], cf(FP, "WT_im"), Alu.mult)
            nc.vector.tensor_tensor(ZTre[:], t1[:], t2[:], Alu.subtract)
            nc.gpsimd.tensor_tensor(t3[:], YreS[:], cf(FP, "WT_im"), Alu.mult)
            nc.gpsimd.tensor_tensor(t4[:], YimS[:], cf(FP, "WT_re"), Alu.mult)
            nc.gpsimd.tensor_tensor(ZTim[:], t3[:], t4[:], Alu.add)

            # ---- stage 2: XT[d,c] = sum_b F128[b,d] ZT[b,c] -------------
            XTre = psfft.tile([128, 64], f32, tag="fftA")
            XTim = psfft.tile([128, 64], f32, tag="fftB")
            nc.tensor.matmul(XTim[:], cb(CB, "F128_im"), ZTre[:], start=True, stop=False)
            nc.tensor.matmul(XTre[:], cb(CB, "F128_re"), ZTre[:], start=True, stop=False)
            nc.tensor.matmul(XTim[:], cb(CB, "F128_re"), ZTim[:], start=False, stop=True)
            nc.tensor.matmul(XTre[:], cb(CB, "F128_imN"), ZTim[:], start=False, stop=True)

            # ---- PSUM->SBUF bf16 copies (stage-3 lhsT) ------------------
            XTreb = work.tile([128, 64], bf16, tag="xtb")
            XTimb = work.tile([128, 64], bf16, tag="xtb2")
            nc.vector.tensor_copy(XTreb[:], XTre[:])
            nc.scalar.copy(XTimb[:], XTim[:])

            # ---- stage 3 (symbol folded): P[c,b] = sum_d X'[d,c]I''[d,b]
            Pre = psfft.tile([64, 128], f32, tag="fftA")
            Pim = psfft.tile([64, 128], f32, tag="fftB")
            nc.tensor.matmul(Pre[:], XTreb[:], cb(CB, "I128f_re"), start=True, stop=False)
            nc.tensor.matmul(Pim[:], XTreb[:], cb(CB, "I128f_im"), start=True, stop=False)
            nc.tensor.matmul(Pre[:], XTimb[:], cb(CB, "I128f_imN"), start=False, stop=True)
            nc.tensor.matmul(Pim[:], XTimb[:], cb(CB, "I128f_re"), start=False, stop=True)

            # ---- inverse twiddle: Q = P * Wi (complex), same split ------
            Qre = work.tile([64, 128], bf16, tag="q")
            Qim = work.tile([64, 128], bf16, tag="q2")
            s1 = work.tile([64, 128], f32, tag="s1")
            s2 = work.tile([64, 128], f32, tag="s2")
            s3 = work.tile([64, 128], f32, tag="s3")
            s4 = work.tile([64, 128], f32, tag="s4")
            PreS = work.tile([64, 128], f32, tag="pres")
            PimS = work.tile([64, 128], f32, tag="pims")
            nc.scalar.copy(PreS[:], Pre[:])
            nc.scalar.copy(PimS[:], Pim[:])
            nc.vector.tensor_tensor(s1[:], Pre[:], cf(FP, "Wi_re")[0:64, :], Alu.mult)
            nc.vector.tensor_tensor(s2[:], Pim[:], cf(FP, "Wi_im")[0:64, :], Alu.mult)
            nc.vector.tensor_tensor(Qre[:], s1[:], s2[:], Alu.subtract)
            nc.gpsimd.tensor_tensor(s3[:], PreS[:], cf(FP, "Wi_im")[0:64, :], Alu.mult)
            nc.gpsimd.tensor_tensor(s4[:], PimS[:], cf(FP, "Wi_re")[0:64, :], Alu.mult)
            nc.gpsimd.tensor_tensor(Qim[:], s3[:], s4[:], Alu.add)

            # ---- stage 4 (doubled): Y2 = [w1grid; w1grid] ---------------
            Y2 = psy.tile([128, 128], f32)
            nc.tensor.matmul(Y2[:], cb(CB, "I64_re2")[0:64, :], Qre[:], start=True, stop=False)
            nc.tensor.matmul(Y2[:], cb(CB, "I64_imN2")[0:64, :], Qim[:], start=False, stop=True)

            # ---- final: res = a*X2 + c*Y2, gathered in one [128,512] ----
            # DVE pairs 0,1 straight from PSUM; Act scales Y2 for pairs
            # 2,3 into SBUF and GpSimd adds the a-halves.
            res = work.tile([128, 512], f32, tag="res")
            tcY = work.tile([128, 256], f32, tag="tcY")
            for j in range(2):
                nc.vector.scalar_tensor_tensor(
                    res[:, 128 * j : 128 * j + 128],
                    Y2[:],
                    acrS[:, 2 * j + 1 : 2 * j + 2],
                    tmpA[:, 128 * j : 128 * j + 128],
                    op0=Alu.mult,
                    op1=Alu.add,
                )
            for j in range(2, 4):
                nc.scalar.activation(
                    tcY[:, 128 * (j - 2) : 128 * (j - 2) + 128],
                    Y2[:],
                    Act.Copy,
                    scale=acrS[:, 2 * j + 1 : 2 * j + 2],
                )
                nc.gpsimd.tensor_tensor(
                    res[:, 128 * j : 128 * j + 128],
                    tcY[:, 128 * (j - 2) : 128 * (j - 2) + 128],
                    tmpA[:, 128 * j : 128 * j + 128],
                    Alu.add,
                )
            # out[b, a, f] with b = 2j + hi <- res[(hi a), (j f)]
            ov = out_d[:].rearrange("(j hi) a f -> (hi a) j f", hi=2)
            rv = res[:].rearrange("p (j f) -> p j f", j=4)
            nc.sync.dma_start(ov[:, 0:2, :], rv[:, 0:2, :])
            nc.sync.dma_start(ov[:, 2:4, :], rv[:, 2:4, :])

    nc.compile()
    return nc


def _get_compiled():
    global _COMPILED
    if _COMPILED is None:
        _COMPILED = _build_nc()
    return _COMPILED


# ---------------------------------------------------------------- entry


def _make_in_maps(u, eigenvectors, eigenvalues):
    u = np.ascontiguousarray(u, np.float32)
    # pure relayout (zero flops): EVr[32s+k, 128t+b] = ev[k, 128(4t+s)+b]
    evr = (
        np.asarray(eigenvectors, np.float32)
        .astype(_BF16)
        .reshape(_K, 16, 4, 128)
        .transpose(2, 0, 1, 3)
        .reshape(128, 2048)
    )
    lamv = np.asarray(eigenvalues, np.float32).astype(_BF16)
    lamb2 = np.zeros((128, 188), _BF16)
    for s in range(4):
        lamb2[32 * s : 32 * s + 32, 60 + s] = lamv
        lamb2[32 * s : 32 * s + 32, 124 + s] = lamv
    a1 = np.ascontiguousarray(np.hstack([lamb2, evr[:, 0:1024]]))
    a2 = np.ascontiguousarray(evr[:, 1024:2048])

    in_maps = []
    for c in range(_NC):
        fp = np.ascontiguousarray(
            np.hstack([u[c * _BS : (c + 1) * _BS].reshape(128, 512), _CF])
        )
        in_maps.append({"a1": a1, "a2": a2, "cb": _CB, "fp": fp})
    return in_maps, None


def _gather(results):
    outs = [results[c]["out"].reshape(_BS, _L) for c in range(_NC)]
    return np.concatenate(outs, axis=0)


def kernel(u, eigenvectors, eigenvalues):
    from concourse.bass_utils import run_bass_kernel_spmd

    nc = _get_compiled()
    in_maps, _ = _make_in_maps(u, eigenvectors, eigenvalues)
    res = run_bass_kernel_spmd(nc, in_maps, core_ids=list(range(_NC)))
    return _gather(res.results)
